# revision 15
# baseline (speedup 1.0000x reference)
"""EMA Vector-Quantiser (VQ codebook) Trainium2 kernel.

Problem: z [8, 4096, 512] tokens, codebook [8192, 512].
Returns (z_q_st, indices, loss_commit, loss_embed, usage, new_codebook,
         new_ema_count, new_ema_sum) matching the jax reference.

Sharding: tokens data-parallel across 8 NeuronCores (4096 tokens/core),
codebook replicated.

Device (per core): raw scores s[n,k] = z[n].cb[k] via bf16 matmuls on the
PE (full-rate, ~437us/core = bf16 roofline for 4096x8192x512), tiled as
32 token-tiles x 2 codebook halves. Scores land in PSUM fp32, are copied
to SBUF as bf16 by the Scalar engine, then the DVE does one
tensor_tensor-max fold (2x bf16 mode) and MAX8/FIND_INDEX8 over the
folded [128, 2048] array: top-8 "blocks" of 2 elements per (token, half).
Only those candidates (~400KB/core) leave the device.

Host: for each token, prunes the 16 candidate blocks with sound interval
bounds (raw bf16 score error + the -0.5*||c||^2 bias range of each
block's 2 members), then re-scores surviving members exactly in fp64 and
picks the argmin-distance index with first-index tie-breaking. Then the
cheap O(N)/O(K*D) tail: gather z_q, histogram, segment-sum, EMA updates,
losses.

Self-contained: shapes hardcoded; no sibling imports.
"""

import numpy as np

B, S, D, K = 8, 4096, 512, 8192
DECAY = 0.99
N_CORES = 8
NT = S // 128           # 32 token-tiles per core
KQ = 4096               # codebook slice per half
NQ = K // KQ            # 2 halves
NPS = KQ // 1024        # 4 psum tiles (2 banks each) per half
BLK = KQ // 2           # 2048 folded blocks per half; block b = {b, b+BLK}
SCORE_ERR = 0.012       # sound bound on |bf16 raw score - exact| (|s|<~2.5)

_cache = {}


def _build():
    import concourse.mybir as mybir
    import concourse.tile as tile
    from concourse import bacc

    F32 = mybir.dt.float32
    BF16 = mybir.dt.bfloat16
    U16 = mybir.dt.uint16

    nc = bacc.Bacc("TRN2", target_bir_lowering=False, debug=False)

    zT = nc.dram_tensor("zT", [D, S], BF16, kind="ExternalInput")
    cbT = nc.dram_tensor("cbT", [D, K], BF16, kind="ExternalInput")
    cand_val = nc.dram_tensor("cand_val", [128, NQ, NT, 8], BF16,
                              kind="ExternalOutput")
    cand_idx = nc.dram_tensor("cand_idx", [128, NQ, NT, 8], U16,
                              kind="ExternalOutput")

    with tile.TileContext(nc) as tc:
        with (
            tc.tile_pool(name="persist", bufs=1) as persist,
            tc.tile_pool(name="cbq", bufs=2) as cbq_pool,
            tc.tile_pool(name="ztt", bufs=3) as zt_pool,
            tc.tile_pool(name="score", bufs=3) as score_pool,
            tc.tile_pool(name="fold", bufs=3) as fold_pool,
            tc.tile_pool(name="psum", bufs=4, space="PSUM") as psum_pool,
        ):
            cv_sb = persist.tile([128, NQ, NT, 8], BF16)
            ci_sb = persist.tile([128, NQ, NT, 8], U16)

            for q in range(NQ):
                cbq = cbq_pool.tile([128, 4, KQ], BF16)
                nc.sync.dma_start(
                    cbq[:],
                    cbT[:, q * KQ : (q + 1) * KQ].rearrange("(c p) n -> p c n", p=128),
                )
                for t in range(NT):
                    ztt = zt_pool.tile([128, 4, 128], BF16)
                    nc.sync.dma_start(
                        ztt[:],
                        zT[:, t * 128 : (t + 1) * 128].rearrange(
                            "(c p) m -> p c m", p=128
                        ),
                    )
                    score = score_pool.tile([128, KQ], BF16)
                    for c in range(NPS):
                        ps = psum_pool.tile([128, 1024], F32)
                        for h in range(2):
                            for d in range(4):
                                nc.tensor.matmul(
                                    ps[:, h * 512 : (h + 1) * 512],
                                    ztt[:, d, :],
                                    cbq[:, d, c * 1024 + h * 512 : c * 1024 + (h + 1) * 512],
                                    start=(d == 0),
                                    stop=(d == 3),
                                )
                        nc.scalar.copy(score[:, c * 1024 : (c + 1) * 1024], ps[:])
                    fold = fold_pool.tile([128, BLK], BF16)
                    nc.vector.tensor_tensor(
                        out=fold[:], in0=score[:, :BLK], in1=score[:, BLK:],
                        op=mybir.AluOpType.max,
                    )
                    nc.vector.max(out=cv_sb[:, q, t, :], in_=fold[:])
                    nc.vector.max_index(ci_sb[:, q, t, :], cv_sb[:, q, t, :], fold[:])

            nc.sync.dma_start(cand_val[:], cv_sb[:])
            nc.sync.dma_start(cand_idx[:], ci_sb[:])
    nc.compile()
    return nc


def _get_nc():
    if "nc" not in _cache:
        _cache["nc"] = _build()
    return _cache["nc"]


def _make_in_maps(z, codebook):
    import ml_dtypes

    bf16 = ml_dtypes.bfloat16
    cbT16 = np.ascontiguousarray(codebook.T).astype(bf16)        # [D, K]
    return [
        {
            "zT": np.ascontiguousarray(z[i].T).astype(bf16),     # [D, S]
            "cbT": cbT16,
        }
        for i in range(N_CORES)
    ]


def _combine_host(cand_val, cand_idx, zf64, cb64, c2half_64):
    """Pick the exact argmin-distance index per token from device candidates.

    cand_val: [128, NQ, NT, 8] bf16 raw-score block maxima.
    cand_idx: [128, NQ, NT, 8] uint16 block index within half (0..BLK-1).
    zf64: [S, D] this core's tokens (fp64). Returns [S] int32 indices.
    """
    # token n = t*128 + p  ->  [t, p, q, slot] -> [S, 16]
    vm = cand_val.astype(np.float64).transpose(2, 0, 1, 3).reshape(S, NQ * 8)
    bi = cand_idx.astype(np.int64).transpose(2, 0, 1, 3)
    # 65535 marks a (rare) unmatched max_index slot; exclude that block.
    invalid = bi >= BLK
    bi = np.where(invalid, 0, bi)
    base = (bi + (np.arange(NQ, dtype=np.int64) * KQ)[None, None, :, None]).reshape(
        S, NQ * 8
    )
    invalid = invalid.reshape(S, NQ * 8)
    k1 = base                    # member 1 global index
    k2 = base + BLK              # member 2 global index

    b1 = c2half_64[k1]
    b2 = c2half_64[k2]
    minb = np.minimum(b1, b2)
    maxb = np.maximum(b1, b2)
    ub = vm - minb + SCORE_ERR   # upper bound on best biased score in block
    lb = vm - maxb - SCORE_ERR   # lower bound achieved by the block's max member
    ub = np.where(invalid, -np.inf, ub)
    lb = np.where(invalid, -np.inf, lb)
    keep = ub >= lb.max(axis=1, keepdims=True)

    tok, blk = np.nonzero(keep)                       # ragged kept blocks
    kk = np.stack([k1[tok, blk], k2[tok, blk]], 1)    # [M, 2]
    tok2 = np.repeat(tok, 2)
    kk = kk.reshape(-1)
    # exact biased scores, chunked fp64
    s = np.empty(kk.shape[0], dtype=np.float64)
    CH = 1 << 14
    for lo in range(0, kk.shape[0], CH):
        hi = min(lo + CH, kk.shape[0])
        s[lo:hi] = (
            np.einsum("md,md->m", zf64[tok2[lo:hi]], cb64[kk[lo:hi]])
            - c2half_64[kk[lo:hi]]
        )
    best = np.full(S, -np.inf)
    np.maximum.at(best, tok2, s)
    is_best = s == best[tok2]
    win = np.full(S, K, dtype=np.int64)
    np.minimum.at(win, tok2[is_best], kk[is_best])
    assert (win < K).all()
    return win.astype(np.int32)


def kernel(z, codebook, ema_count, ema_sum):
    from concourse.bass_utils import run_bass_kernel_spmd

    z = np.asarray(z, dtype=np.float32)
    codebook = np.asarray(codebook, dtype=np.float32)
    ema_count = np.asarray(ema_count, dtype=np.float32)
    ema_sum = np.asarray(ema_sum, dtype=np.float32)

    cb64 = codebook.astype(np.float64)
    c2half_64 = 0.5 * (cb64 * cb64).sum(1)

    in_maps = _make_in_maps(z, codebook)
    res = run_bass_kernel_spmd(_get_nc(), in_maps, core_ids=list(range(N_CORES)))
    results = res.results

    z64 = z.reshape(-1, D).astype(np.float64)
    idx = np.empty((B, S), dtype=np.int32)
    for i in range(N_CORES):
        idx[i] = _combine_host(
            np.asarray(results[i]["cand_val"]),
            results[i]["cand_idx"],
            z64[i * S : (i + 1) * S],
            cb64,
            c2half_64,
        )

    idx_flat = idx.reshape(-1)
    z_q = codebook[idx_flat].reshape(B, S, D)

    # ---- host-side glue (cheap O(N), O(K*D)) ----
    counts = np.bincount(idx_flat, minlength=K).astype(np.float64)    # [K]

    order = np.argsort(idx_flat, kind="stable")
    sorted_idx = idx_flat[order]
    sorted_z = z64[order]
    sums = np.zeros((K, D), dtype=np.float64)
    uniq, starts = np.unique(sorted_idx, return_index=True)
    sums[uniq] = np.add.reduceat(sorted_z, starts, axis=0)

    diff = z_q.reshape(-1, D).astype(np.float64) - z64
    loss = float(np.einsum("ij,ij->", diff, diff) / diff.size)
    loss_commit = np.float32(loss)
    loss_embed = np.float32(loss)

    total = counts.sum()
    usage = (counts / total if total > 0 else counts).astype(np.float32)

    new_ema_count = (
        ema_count.astype(np.float64) * DECAY + counts * (1.0 - DECAY)
    ).astype(np.float32)
    new_ema_sum = (
        ema_sum.astype(np.float64) * DECAY + sums * (1.0 - DECAY)
    ).astype(np.float32)
    denom = np.clip(new_ema_count.astype(np.float64)[:, None], 1.0, None)
    new_codebook = (new_ema_sum.astype(np.float64) / denom).astype(np.float32)

    return (
        z_q,                      # z_q_st == z_q in forward values
        idx,
        loss_commit,
        loss_embed,
        usage,
        new_codebook,
        new_ema_count,
        new_ema_sum,
    )


# revision 16
# speedup vs baseline: 1.0178x; 1.0178x over previous
"""EMA Vector-Quantiser (VQ codebook) Trainium2 kernel.

Problem: z [8, 4096, 512] tokens, codebook [8192, 512].
Returns (z_q_st, indices, loss_commit, loss_embed, usage, new_codebook,
         new_ema_count, new_ema_sum) matching the jax reference.

Sharding: tokens data-parallel across 8 NeuronCores (4096 tokens/core),
codebook replicated.

Device (per core): raw scores s[n,k] = z[n].cb[k] via bf16 matmuls on the
PE (full-rate, ~437us/core = bf16 roofline for 4096x8192x512 MACs),
tiled as 32 token-tiles x 2 codebook halves x 2 sub-halves of 2048.
Scores land in PSUM fp32 (4-bank tiles), are copied to SBUF as bf16 by
the Scalar engine, then the DVE does one tensor_tensor-max fold (2x bf16
mode) and MAX8/FIND_INDEX8 over the folded [128, 1024] array: top-8
"blocks" of 2 elements per (token, sub-half). Only those candidates
(~800KB/core) leave the device.

Host: for each token, prunes the 32 candidate blocks with sound interval
bounds (raw bf16 score error + the -0.5*||c||^2 bias range of each
block's 2 members), then re-scores surviving members exactly in fp64 and
picks the argmin-distance index with first-index tie-breaking. Then the
cheap O(N)/O(K*D) tail: gather z_q, histogram, segment-sum, EMA updates,
losses.

Self-contained: shapes hardcoded; no sibling imports.
"""

import numpy as np

B, S, D, K = 8, 4096, 512, 8192
DECAY = 0.99
N_CORES = 8
NT = S // 128           # 32 token-tiles per core
KQ = 4096               # codebook slice per half
NQ = K // KQ            # 2 halves
HB = 1024               # fold block stride: block b = {b, b+HB} in a sub-half
SCORE_ERR = 0.012       # sound bound on |bf16 raw score - exact| (|s|<~2.5)

_cache = {}


def _build():
    import concourse.mybir as mybir
    import concourse.tile as tile
    from concourse import bacc

    F32 = mybir.dt.float32
    BF16 = mybir.dt.bfloat16
    U16 = mybir.dt.uint16

    nc = bacc.Bacc("TRN2", target_bir_lowering=False, debug=False)

    zT = nc.dram_tensor("zT", [D, S], BF16, kind="ExternalInput")
    cbT = nc.dram_tensor("cbT", [D, K], BF16, kind="ExternalInput")
    cand_val = nc.dram_tensor("cand_val", [128, NQ, NT, 2, 8], BF16,
                              kind="ExternalOutput")
    cand_idx = nc.dram_tensor("cand_idx", [128, NQ, NT, 2, 8], U16,
                              kind="ExternalOutput")

    with tile.TileContext(nc) as tc:
        with (
            tc.tile_pool(name="persist", bufs=1) as persist,
            tc.tile_pool(name="cbq", bufs=2) as cbq_pool,
            tc.tile_pool(name="ztt", bufs=3) as zt_pool,
            tc.tile_pool(name="score", bufs=3) as score_pool,
            tc.tile_pool(name="fold", bufs=3) as fold_pool,
            tc.tile_pool(name="psum", bufs=2, space="PSUM") as psum_pool,
        ):
            cv_sb = persist.tile([128, NQ, NT, 2, 8], BF16)
            ci_sb = persist.tile([128, NQ, NT, 2, 8], U16)

            for q in range(NQ):
                cbq = cbq_pool.tile([128, 4, KQ], BF16)
                # split the 4MB codebook-half load so compute starts early
                cbr = cbT[:, q * KQ : (q + 1) * KQ].rearrange(
                    "(c p) n -> p c n", p=128
                )
                nc.sync.dma_start(cbq[:, :, 0:1024], cbr[:, :, 0:1024])
                for t in range(NT):
                    ztt = zt_pool.tile([128, 4, 128], BF16)
                    nc.sync.dma_start(
                        ztt[:],
                        zT[:, t * 128 : (t + 1) * 128].rearrange(
                            "(c p) m -> p c m", p=128
                        ),
                    )
                    if t == 0:
                        for j in range(1, 4):
                            nc.sync.dma_start(
                                cbq[:, :, j * 1024 : (j + 1) * 1024],
                                cbr[:, :, j * 1024 : (j + 1) * 1024],
                            )
                    score = score_pool.tile([128, KQ], BF16)
                    for h in range(2):
                        ps = psum_pool.tile([128, 2048], F32)
                        for g in range(4):
                            for d in range(4):
                                nc.tensor.matmul(
                                    ps[:, g * 512 : (g + 1) * 512],
                                    ztt[:, d, :],
                                    cbq[:, d,
                                        h * 2048 + g * 512 : h * 2048 + (g + 1) * 512],
                                    start=(d == 0),
                                    stop=(d == 3),
                                )
                        nc.scalar.copy(score[:, h * 2048 : (h + 1) * 2048], ps[:])
                        fold = fold_pool.tile([128, HB], BF16)
                        nc.vector.tensor_tensor(
                            out=fold[:],
                            in0=score[:, h * 2048 : h * 2048 + HB],
                            in1=score[:, h * 2048 + HB : (h + 1) * 2048],
                            op=mybir.AluOpType.max,
                        )
                        nc.vector.max(out=cv_sb[:, q, t, h, :], in_=fold[:])
                        nc.vector.max_index(
                            ci_sb[:, q, t, h, :], cv_sb[:, q, t, h, :], fold[:]
                        )

            nc.sync.dma_start(cand_val[:], cv_sb[:])
            nc.sync.dma_start(cand_idx[:], ci_sb[:])
    nc.compile()
    return nc


def _get_nc():
    if "nc" not in _cache:
        _cache["nc"] = _build()
    return _cache["nc"]


def _make_in_maps(z, codebook):
    import ml_dtypes

    bf16 = ml_dtypes.bfloat16
    cbT16 = np.ascontiguousarray(codebook.T).astype(bf16)        # [D, K]
    return [
        {
            "zT": np.ascontiguousarray(z[i].T).astype(bf16),     # [D, S]
            "cbT": cbT16,
        }
        for i in range(N_CORES)
    ]


def _combine_host(cand_val, cand_idx, zf64, cb64, c2half_64):
    """Pick the exact argmin-distance index per token from device candidates.

    cand_val: [128, NQ, NT, 2, 8] bf16 raw-score block maxima.
    cand_idx: [128, NQ, NT, 2, 8] uint16 block index in sub-half (0..HB-1).
    zf64: [S, D] this core's tokens (fp64). Returns [S] int32 indices.
    """
    NCAND = NQ * 2 * 8
    # token n = t*128 + p  ->  [t, p, q, h, slot] -> [S, NCAND]
    vm = cand_val.astype(np.float64).transpose(2, 0, 1, 3, 4).reshape(S, NCAND)
    bi = cand_idx.astype(np.int64).transpose(2, 0, 1, 3, 4)  # [t, p, q, h, 8]
    # 65535 marks a (rare) unmatched max_index slot; exclude that block.
    invalid = bi >= HB
    bi = np.where(invalid, 0, bi)
    off = (
        (np.arange(NQ, dtype=np.int64) * KQ)[None, None, :, None, None]
        + (np.arange(2, dtype=np.int64) * 2048)[None, None, None, :, None]
    )
    base = (bi + off).reshape(S, NCAND)
    invalid = invalid.reshape(S, NCAND)
    k1 = base                    # member 1 global index
    k2 = base + HB               # member 2 global index

    b1 = c2half_64[k1]
    b2 = c2half_64[k2]
    minb = np.minimum(b1, b2)
    maxb = np.maximum(b1, b2)
    ub = vm - minb + SCORE_ERR   # upper bound on best biased score in block
    lb = vm - maxb - SCORE_ERR   # lower bound achieved by the block's max member
    ub = np.where(invalid, -np.inf, ub)
    lb = np.where(invalid, -np.inf, lb)
    keep = ub >= lb.max(axis=1, keepdims=True)

    tok, blk = np.nonzero(keep)                       # ragged kept blocks
    kk = np.stack([k1[tok, blk], k2[tok, blk]], 1)    # [M, 2]
    tok2 = np.repeat(tok, 2)
    kk = kk.reshape(-1)
    # exact biased scores, chunked fp64
    s = np.empty(kk.shape[0], dtype=np.float64)
    CH = 1 << 14
    for lo in range(0, kk.shape[0], CH):
        hi = min(lo + CH, kk.shape[0])
        s[lo:hi] = (
            np.einsum("md,md->m", zf64[tok2[lo:hi]], cb64[kk[lo:hi]])
            - c2half_64[kk[lo:hi]]
        )
    best = np.full(S, -np.inf)
    np.maximum.at(best, tok2, s)
    is_best = s == best[tok2]
    win = np.full(S, K, dtype=np.int64)
    np.minimum.at(win, tok2[is_best], kk[is_best])
    assert (win < K).all()
    return win.astype(np.int32)


def kernel(z, codebook, ema_count, ema_sum):
    from concourse.bass_utils import run_bass_kernel_spmd

    z = np.asarray(z, dtype=np.float32)
    codebook = np.asarray(codebook, dtype=np.float32)
    ema_count = np.asarray(ema_count, dtype=np.float32)
    ema_sum = np.asarray(ema_sum, dtype=np.float32)

    cb64 = codebook.astype(np.float64)
    c2half_64 = 0.5 * (cb64 * cb64).sum(1)

    in_maps = _make_in_maps(z, codebook)
    res = run_bass_kernel_spmd(_get_nc(), in_maps, core_ids=list(range(N_CORES)))
    results = res.results

    z64 = z.reshape(-1, D).astype(np.float64)
    idx = np.empty((B, S), dtype=np.int32)
    for i in range(N_CORES):
        idx[i] = _combine_host(
            np.asarray(results[i]["cand_val"]),
            results[i]["cand_idx"],
            z64[i * S : (i + 1) * S],
            cb64,
            c2half_64,
        )

    idx_flat = idx.reshape(-1)
    z_q = codebook[idx_flat].reshape(B, S, D)

    # ---- host-side glue (cheap O(N), O(K*D)) ----
    counts = np.bincount(idx_flat, minlength=K).astype(np.float64)    # [K]

    order = np.argsort(idx_flat, kind="stable")
    sorted_idx = idx_flat[order]
    sorted_z = z64[order]
    sums = np.zeros((K, D), dtype=np.float64)
    uniq, starts = np.unique(sorted_idx, return_index=True)
    sums[uniq] = np.add.reduceat(sorted_z, starts, axis=0)

    diff = z_q.reshape(-1, D).astype(np.float64) - z64
    loss = float(np.einsum("ij,ij->", diff, diff) / diff.size)
    loss_commit = np.float32(loss)
    loss_embed = np.float32(loss)

    total = counts.sum()
    usage = (counts / total if total > 0 else counts).astype(np.float32)

    new_ema_count = (
        ema_count.astype(np.float64) * DECAY + counts * (1.0 - DECAY)
    ).astype(np.float32)
    new_ema_sum = (
        ema_sum.astype(np.float64) * DECAY + sums * (1.0 - DECAY)
    ).astype(np.float32)
    denom = np.clip(new_ema_count.astype(np.float64)[:, None], 1.0, None)
    new_codebook = (new_ema_sum.astype(np.float64) / denom).astype(np.float32)

    return (
        z_q,                      # z_q_st == z_q in forward values
        idx,
        loss_commit,
        loss_embed,
        usage,
        new_codebook,
        new_ema_count,
        new_ema_sum,
    )
